# revision 24
# baseline (speedup 1.0000x reference)
"""Trainium2 Bass kernel: masked-softmax attention pooling.

reference semantics (per batch b):
    energy[s] = sum_d key[b,s,d] * token[b,d]            # [S]
    w         = softmax(energy)                          # over all S
    w[s >= lens[b]] = 1e-9                               # mask AFTER softmax
    out[d]    = sum_s value[b,s,d] * w[s]                # [D]

Sharding: pure data parallel over batch. 8 cores x 4 batches each.

Device layout: position s = p*CPP + c  (p = SBUF partition, c = free-dim
chunk).  key/value are staged to fp16 on the host (error budget measured:
~3e-3 relative, tolerance 2e-2) and loaded as [128, CPP/2, D] half-batch
tiles where each partition reads one contiguous run of DRAM (line-rate
DMA; 16.6 MB/core total vs 32.6 MB in fp32).

Per batch on device (software-pipelined: batch b+1's load+energy phase is
emitted before batch b's softmax/context so per-engine FIFOs don't
head-of-line block on the softmax latency chain):
  - energy: one in-place fp16 tensor_mul per half (token broadcast via
    step-0 AP, 2x DVE mode) + d-reduction split between DVE (one 3D-AP
    reduce_sum) and ScalarE (Copy with fused accum) to balance engines
  - softmax: reduce_max -> gpsimd.partition_all_reduce(max) -> ACT Exp
    (bias=-M, out=fp16 w, fused sum accum) -> partition_all_reduce(add)
    -> reciprocal; mask applied with copy_predicated (fill underflows
    fp16 to 0; the 1e-9*sum(masked v) term is ~1e-7 relative)
  - context: CPP fp16 PE matmuls (1 cyc/row), lhsT = w[:,c] (M=1),
    rhs = value chunk (N=D), accumulated in one PSUM bank; 1/Z applied
    on the final [1, D] PSUM->SBUF copy (keeps Z off the matmul path)
"""

import numpy as np
from contextlib import ExitStack

import concourse.bass as bass
import concourse.tile as tile
from concourse import bacc, mybir, bass_isa
from concourse import bass_utils

B, S, D = 32, 4096, 256
NCORES = 8
BPC = B // NCORES        # batches per core
P = 128                  # SBUF partitions
CPP = S // P             # free-dim chunks per batch (32); s = p*CPP + c
MASK_FILL = 1e-9
F32 = mybir.dt.float32


def emit(tc, key, val, tok, msk, out, bpc, s, d):
    """Emit the per-core program.  key/val: [bpc, s, d], tok: [bpc, P, d],
    msk: [bpc, P, cpp] (1.0 where masked), out: [bpc, d]."""
    nc = tc.nc
    cpp = s // P
    with ExitStack() as ctx:
        kpool = ctx.enter_context(tc.tile_pool(name="kpool", bufs=4))
        vpool = ctx.enter_context(tc.tile_pool(name="vpool", bufs=8))
        tpool = ctx.enter_context(tc.tile_pool(name="tpool", bufs=2))
        spool = ctx.enter_context(tc.tile_pool(name="spool", bufs=4))
        cpool = ctx.enter_context(tc.tile_pool(name="cpool", bufs=1))
        pspool = ctx.enter_context(tc.tile_pool(name="pspool", bufs=4, space="PSUM"))

        BF16 = mybir.dt.float16  # fp16: 10-bit mantissa, 1 cyc/row on PE
        fillc = cpool.tile([P, cpp], BF16)
        nc.vector.memset(fillc[:], MASK_FILL)
        dump = cpool.tile([P, d], BF16)

        HALVES = 2
        cph = cpp // HALVES  # chunks per half
        state = {}

        def load_energy(b):
            tokt = tpool.tile([P, d], BF16)
            nc.sync.dma_start(tokt[:], tok[b])
            maskt = spool.tile([P, cpp], mybir.dt.uint8)
            nc.sync.dma_start(maskt[:], msk[b])

            # energy E[p, c] = sum_d key[s, :] * token   (s = p*cpp + c)
            # one in-place fp16 multiply (token broadcast via step-0 AP) per
            # half; d-reduction split between DVE (3D-AP reduce) and ScalarE
            # (Copy + accum) to balance engine time.
            E = spool.tile([P, cpp], F32)
            vth = []
            key3 = key[b].rearrange("(p h c) d -> h p c d", p=P, h=HALVES)
            val3 = val[b].rearrange("(p h c) d -> h p c d", p=P, h=HALVES)
            tok_b = tokt[:].rearrange("p (c d) -> p c d", c=1).broadcast_to(
                [P, cph, d]
            )
            for h in range(HALVES):
                kt = kpool.tile([P, cph, d], BF16)
                nc.sync.dma_start(kt[:], key3[h])
                vt = vpool.tile([P, cph, d], BF16)
                nc.sync.dma_start(vt[:], val3[h])
                vth.append(vt)
                nc.vector.tensor_mul(kt[:], kt[:], tok_b)
                red_dve = min(10, cph)  # DVE/ACT reduce split balance
                nc.vector.reduce_sum(
                    E[:, h * cph : h * cph + red_dve],
                    kt[:, 0:red_dve],
                    axis=mybir.AxisListType.X,
                )
                for c in range(red_dve, cph):
                    nc.scalar.activation(
                        dump[:],
                        kt[:, c],
                        mybir.ActivationFunctionType.Copy,
                        accum_out=E[:, h * cph + c : h * cph + c + 1],
                    )
            state[b] = (E, maskt, vth)

        def finish(b):
            E, maskt, vth = state.pop(b)
            # softmax over all s
            m1 = spool.tile([P, 1], F32)
            nc.vector.reduce_max(m1[:], E[:], axis=mybir.AxisListType.X)
            mb = spool.tile([P, 1], F32)
            nc.gpsimd.partition_all_reduce(
                mb[:], m1[:], channels=P, reduce_op=bass_isa.ReduceOp.max
            )
            negm = spool.tile([P, 1], F32)
            nc.vector.tensor_scalar_mul(negm[:], mb[:], -1.0)
            s1 = spool.tile([P, 1], F32)
            w = spool.tile([P, cpp], BF16)
            nc.scalar.activation(
                w[:],
                E[:],
                mybir.ActivationFunctionType.Exp,
                bias=negm[:],
                scale=1.0,
                accum_out=s1[:],
            )
            zb = spool.tile([P, 1], F32)
            nc.gpsimd.partition_all_reduce(
                zb[:], s1[:], channels=P, reduce_op=bass_isa.ReduceOp.add
            )
            zi = spool.tile([P, 1], F32)
            nc.vector.reciprocal(zi[:], zb[:])
            # unnormalized masked weights; 1/Z is applied to the [1, d]
            # context instead (the 1e-9 fill underflows fp16 -> 0; its
            # contribution is ~1e-7 relative)
            nc.vector.copy_predicated(w[:], maskt[:], fillc[:])

            # context[d] = sum_s w[s] * value[s, d]  (fp16 matmul, 1 cyc/row)
            cps = pspool.tile([1, d], F32)
            for c in range(cpp):
                nc.tensor.matmul(
                    cps[:],
                    lhsT=w[:, c : c + 1],
                    rhs=vth[c // cph][:, c % cph],
                    start=(c == 0),
                    stop=(c == cpp - 1),
                )
            ctx_s = spool.tile([1, d], F32)
            nc.vector.tensor_scalar_mul(ctx_s[:], cps[:], zi[0:1])
            nc.sync.dma_start(out[b], ctx_s[:])

        # software pipeline: batch b's softmax/context is emitted after
        # batch b+1's load+energy so per-engine FIFOs never head-of-line
        # block on the cross-engine softmax latency chain.
        for b in range(bpc):
            load_energy(b)
            if b >= 1:
                finish(b - 1)
        finish(bpc - 1)


def build(bpc=BPC, s=S, d=D, num_devices=NCORES):
    nc = bacc.Bacc(
        "TRN2",
        target_bir_lowering=False,
        debug=False,
        enable_asserts=False,
        num_devices=num_devices,
    )
    cpp = s // P
    key_d = nc.dram_tensor("key", [bpc, s, d], mybir.dt.float16, kind="ExternalInput")
    val_d = nc.dram_tensor("value", [bpc, s, d], mybir.dt.float16, kind="ExternalInput")
    tok_d = nc.dram_tensor("token_rep", [bpc, P, d], mybir.dt.float16, kind="ExternalInput")
    msk_d = nc.dram_tensor("maskf", [bpc, P, cpp], mybir.dt.uint8, kind="ExternalInput")
    out_d = nc.dram_tensor("out", [bpc, d], F32, kind="ExternalOutput")
    with tile.TileContext(nc) as tc:
        emit(tc, key_d.ap(), val_d.ap(), tok_d.ap(), msk_d.ap(), out_d.ap(), bpc, s, d)
    nc.compile()
    return nc


def make_in_maps(key, value, token, lens, bpc=BPC, ncores=NCORES):
    """Shard the full inputs over cores and build per-core host tensors."""
    s = key.shape[1]
    cpp = s // P
    key = np.ascontiguousarray(key, dtype=np.float16)
    value = np.ascontiguousarray(value, dtype=np.float16)
    token = np.asarray(token, dtype=np.float32)
    lens = np.asarray(lens).astype(np.int64)
    sidx = (np.arange(P)[:, None] * cpp + np.arange(cpp)[None, :])  # [P, cpp]
    in_maps = []
    for core in range(ncores):
        b0 = core * bpc
        lb = lens[b0 : b0 + bpc]
        maskf = (sidx[None, :, :] >= lb[:, None, None]).astype(np.uint8)
        tok_rep = np.ascontiguousarray(
            np.broadcast_to(token[b0 : b0 + bpc, None, :], (bpc, P, token.shape[1]))
        ).astype(np.float16)
        in_maps.append(
            {
                "key": key[b0 : b0 + bpc],
                "value": value[b0 : b0 + bpc],
                "token_rep": tok_rep,
                "maskf": maskf,
            }
        )
    return in_maps


_NC_CACHE = None


def _get_nc():
    global _NC_CACHE
    if _NC_CACHE is None:
        _NC_CACHE = build()
    return _NC_CACHE


def run(key, value, token, lens, trace=False, **kwargs):
    """Run on 8 NeuronCores; returns (output [B, D], BassKernelResults)."""
    nc = _get_nc()
    in_maps = make_in_maps(key, value, token, lens)
    res = bass_utils.run_bass_kernel_spmd(
        nc, in_maps, core_ids=list(range(NCORES)), trace=trace, **kwargs
    )
    outs = [res.results[i]["out"] for i in range(NCORES)]
    full = np.concatenate(outs, axis=0).astype(np.float32)
    return full, res


def kernel(key, value, token, lens):
    full, _ = run(key, value, token, lens)
    return full


# revision 25
# speedup vs baseline: 1.0280x; 1.0280x over previous
"""Trainium2 Bass kernel: masked-softmax attention pooling.

reference semantics (per batch b):
    energy[s] = sum_d key[b,s,d] * token[b,d]            # [S]
    w         = softmax(energy)                          # over all S
    w[s >= lens[b]] = 1e-9                               # mask AFTER softmax
    out[d]    = sum_s value[b,s,d] * w[s]                # [D]

Sharding: pure data parallel over batch. 8 cores x 4 batches each.

Device layout: position s = p*CPP + c  (p = SBUF partition, c = free-dim
chunk).  key/value are staged to fp16 on the host (error budget measured:
~3e-3 relative, tolerance 2e-2) and loaded as [128, CPP/2, D] half-batch
tiles where each partition reads one contiguous run of DRAM (line-rate
DMA; 16.6 MB/core total vs 32.6 MB in fp32).

Per batch on device (software-pipelined: batch b+1's load+energy phase is
emitted before batch b's softmax/context so per-engine FIFOs don't
head-of-line block on the softmax latency chain):
  - energy: one in-place fp16 tensor_mul per half (token broadcast via
    step-0 AP, 2x DVE mode) + d-reduction split between DVE (one 3D-AP
    reduce_sum) and ScalarE (Copy with fused accum) to balance engines
  - softmax: reduce_max -> gpsimd.partition_all_reduce(max) -> ACT Exp
    (bias=-M, out=fp16 w, fused sum accum) -> partition_all_reduce(add)
    -> reciprocal; mask applied with copy_predicated (fill underflows
    fp16 to 0; the 1e-9*sum(masked v) term is ~1e-7 relative)
  - context: CPP fp16 PE matmuls (1 cyc/row), lhsT = w[:,c] (M=1),
    rhs = value chunk (N=D), accumulated in one PSUM bank; 1/Z applied
    on the final [1, D] PSUM->SBUF copy (keeps Z off the matmul path)
"""

import numpy as np
from contextlib import ExitStack

import concourse.bass as bass
import concourse.tile as tile
from concourse import bacc, mybir, bass_isa
from concourse import bass_utils

B, S, D = 32, 4096, 256
NCORES = 8
BPC = B // NCORES        # batches per core
P = 128                  # SBUF partitions
CPP = S // P             # free-dim chunks per batch (32); s = p*CPP + c
MASK_FILL = 1e-9
F32 = mybir.dt.float32


def emit(tc, key, val, tok, msk, out, bpc, s, d):
    """Emit the per-core program.  key/val: [bpc, s, d], tok: [bpc, P, d],
    msk: [bpc, P, cpp] (1.0 where masked), out: [bpc, d]."""
    nc = tc.nc
    cpp = s // P
    with ExitStack() as ctx:
        kpool = ctx.enter_context(tc.tile_pool(name="kpool", bufs=6))
        vpool = ctx.enter_context(tc.tile_pool(name="vpool", bufs=8))
        tpool = ctx.enter_context(tc.tile_pool(name="tpool", bufs=2))
        spool = ctx.enter_context(tc.tile_pool(name="spool", bufs=4))
        cpool = ctx.enter_context(tc.tile_pool(name="cpool", bufs=1))
        pspool = ctx.enter_context(tc.tile_pool(name="pspool", bufs=4, space="PSUM"))

        BF16 = mybir.dt.float16  # fp16: 10-bit mantissa, 1 cyc/row on PE
        fillc = cpool.tile([P, cpp], BF16)
        nc.vector.memset(fillc[:], MASK_FILL)
        dump = cpool.tile([P, d], BF16)

        HALVES = 2
        cph = cpp // HALVES  # chunks per half
        state = {}

        def load_energy(b):
            tokt = tpool.tile([P, d], BF16)
            nc.sync.dma_start(tokt[:], tok[b])
            maskt = spool.tile([P, cpp], mybir.dt.uint8)
            nc.sync.dma_start(maskt[:], msk[b])

            # energy E[p, c] = sum_d key[s, :] * token   (s = p*cpp + c)
            # one in-place fp16 multiply (token broadcast via step-0 AP) per
            # half; d-reduction split between DVE (3D-AP reduce) and ScalarE
            # (Copy + accum) to balance engine time.
            E = spool.tile([P, cpp], F32)
            vth = []
            key3 = key[b].rearrange("(p h c) d -> h p c d", p=P, h=HALVES)
            val3 = val[b].rearrange("(p h c) d -> h p c d", p=P, h=HALVES)
            tok_b = tokt[:].rearrange("p (c d) -> p c d", c=1).broadcast_to(
                [P, cph, d]
            )
            for h in range(HALVES):
                kt = kpool.tile([P, cph, d], BF16)
                nc.sync.dma_start(kt[:], key3[h])
                vt = vpool.tile([P, cph, d], BF16)
                nc.sync.dma_start(vt[:], val3[h])
                vth.append(vt)
                nc.vector.tensor_mul(kt[:], kt[:], tok_b)
                red_dve = min(10, cph)  # DVE/ACT reduce split balance
                nc.vector.reduce_sum(
                    E[:, h * cph : h * cph + red_dve],
                    kt[:, 0:red_dve],
                    axis=mybir.AxisListType.X,
                )
                for c in range(red_dve, cph):
                    nc.scalar.activation(
                        dump[:],
                        kt[:, c],
                        mybir.ActivationFunctionType.Copy,
                        accum_out=E[:, h * cph + c : h * cph + c + 1],
                    )
            state[b] = (E, maskt, vth)

        def finish(b):
            E, maskt, vth = state.pop(b)
            # softmax over all s
            m1 = spool.tile([P, 1], F32)
            nc.vector.reduce_max(m1[:], E[:], axis=mybir.AxisListType.X)
            mb = spool.tile([P, 1], F32)
            nc.gpsimd.partition_all_reduce(
                mb[:], m1[:], channels=P, reduce_op=bass_isa.ReduceOp.max
            )
            negm = spool.tile([P, 1], F32)
            nc.scalar.mul(negm[:], mb[:], -1.0)
            s1 = spool.tile([P, 1], F32)
            w = spool.tile([P, cpp], BF16)
            nc.scalar.activation(
                w[:],
                E[:],
                mybir.ActivationFunctionType.Exp,
                bias=negm[:],
                scale=1.0,
                accum_out=s1[:],
            )
            zb = spool.tile([P, 1], F32)
            nc.gpsimd.partition_all_reduce(
                zb[:], s1[:], channels=P, reduce_op=bass_isa.ReduceOp.add
            )
            zi = spool.tile([P, 1], F32)
            nc.vector.reciprocal(zi[:], zb[:])
            # unnormalized masked weights; 1/Z is applied to the [1, d]
            # context instead (the 1e-9 fill underflows fp16 -> 0; its
            # contribution is ~1e-7 relative)
            nc.vector.copy_predicated(w[:], maskt[:], fillc[:])

            # context[d] = sum_s w[s] * value[s, d]  (fp16 matmul, 1 cyc/row)
            cps = pspool.tile([1, d], F32)
            for c in range(cpp):
                nc.tensor.matmul(
                    cps[:],
                    lhsT=w[:, c : c + 1],
                    rhs=vth[c // cph][:, c % cph],
                    start=(c == 0),
                    stop=(c == cpp - 1),
                )
            ctx_s = spool.tile([1, d], F32)
            nc.scalar.mul(ctx_s[:], cps[:], zi[0:1])
            nc.sync.dma_start(out[b], ctx_s[:])

        # software pipeline: batch b's softmax/context is emitted after
        # batch b+1's load+energy so per-engine FIFOs never head-of-line
        # block on the cross-engine softmax latency chain.
        for b in range(bpc):
            load_energy(b)
            if b >= 1:
                finish(b - 1)
        finish(bpc - 1)


def build(bpc=BPC, s=S, d=D, num_devices=NCORES):
    nc = bacc.Bacc(
        "TRN2",
        target_bir_lowering=False,
        debug=False,
        enable_asserts=False,
        num_devices=num_devices,
    )
    cpp = s // P
    key_d = nc.dram_tensor("key", [bpc, s, d], mybir.dt.float16, kind="ExternalInput")
    val_d = nc.dram_tensor("value", [bpc, s, d], mybir.dt.float16, kind="ExternalInput")
    tok_d = nc.dram_tensor("token_rep", [bpc, P, d], mybir.dt.float16, kind="ExternalInput")
    msk_d = nc.dram_tensor("maskf", [bpc, P, cpp], mybir.dt.uint8, kind="ExternalInput")
    out_d = nc.dram_tensor("out", [bpc, d], F32, kind="ExternalOutput")
    with tile.TileContext(nc) as tc:
        emit(tc, key_d.ap(), val_d.ap(), tok_d.ap(), msk_d.ap(), out_d.ap(), bpc, s, d)
    nc.compile()
    return nc


def make_in_maps(key, value, token, lens, bpc=BPC, ncores=NCORES):
    """Shard the full inputs over cores and build per-core host tensors."""
    s = key.shape[1]
    cpp = s // P
    key = np.ascontiguousarray(key, dtype=np.float16)
    value = np.ascontiguousarray(value, dtype=np.float16)
    token = np.asarray(token, dtype=np.float32)
    lens = np.asarray(lens).astype(np.int64)
    sidx = (np.arange(P)[:, None] * cpp + np.arange(cpp)[None, :])  # [P, cpp]
    in_maps = []
    for core in range(ncores):
        b0 = core * bpc
        lb = lens[b0 : b0 + bpc]
        maskf = (sidx[None, :, :] >= lb[:, None, None]).astype(np.uint8)
        tok_rep = np.ascontiguousarray(
            np.broadcast_to(token[b0 : b0 + bpc, None, :], (bpc, P, token.shape[1]))
        ).astype(np.float16)
        in_maps.append(
            {
                "key": key[b0 : b0 + bpc],
                "value": value[b0 : b0 + bpc],
                "token_rep": tok_rep,
                "maskf": maskf,
            }
        )
    return in_maps


_NC_CACHE = None


def _get_nc():
    global _NC_CACHE
    if _NC_CACHE is None:
        _NC_CACHE = build()
    return _NC_CACHE


def run(key, value, token, lens, trace=False, **kwargs):
    """Run on 8 NeuronCores; returns (output [B, D], BassKernelResults)."""
    nc = _get_nc()
    in_maps = make_in_maps(key, value, token, lens)
    res = bass_utils.run_bass_kernel_spmd(
        nc, in_maps, core_ids=list(range(NCORES)), trace=trace, **kwargs
    )
    outs = [res.results[i]["out"] for i in range(NCORES)]
    full = np.concatenate(outs, axis=0).astype(np.float32)
    return full, res


def kernel(key, value, token, lens):
    full, _ = run(key, value, token, lens)
    return full
